# revision 45
# baseline (speedup 1.0000x reference)
"""Multi-head attention (B=2, S=2048, D=1024, H=16) on 8 TRN2 NeuronCores.

Sharding (Megatron-style): heads are tensor-parallel across the 8 cores
(2 heads each, batch replicated as part of each core's work).
Wq/Wk/Wv are column-parallel (each core gets its heads' 128 output rows),
Wo is row-parallel (each core gets the matching 128 input columns); each
core computes a full-shape partial of the output projection and the host
sums the 8 partials (the row-parallel all-reduce, done at unshard time).

Per-core kernel (fp16 matmuls, fp32 PSUM), scheduled to overlap the
scalar-engine softmax exp (the critical resource) with the tensor engine:

  - x is pre-arranged on the host so each 512-query chunk is one
    contiguous-per-partition DMA; chunks arrive in compute order so the
    first projection starts ~16us in (vs ~27us with whole-slab loads;
    the remaining latency is the DMA path's own cold ramp).
  - a short burst of dummy matmuls at t=0 warms the PE HAM clock gate
    while the first DMA is in flight.
  - V is computed directly in [seq, head-dim] layout (x^T slab as the
    stationary operand), so no PE transposes; a ones column makes the PV
    matmul also produce the softmax denominator (PSUM row 64).
  - scores are co-issued per head pair on disjoint PE row groups; exp
    reads score PSUM [128, 1024] per key block and writes fp16 P.
  - the softmax normalization stages the PV accumulator out of PSUM on
    the scalar engine (it lands right behind the chunk's last exp in the
    ACT queue, so the PSUM banks free deterministically fast), then
    normalizes on the vector engine with a GPSIMD partition-broadcast of
    the denominator row.
  - batch 1's projections fill the tensor engine during batch 0's
    attention; the output projection fills during batch 1's attention
    (emission-interleaved via _Filler so the per-engine queues overlap),
    with a couple of blocks held back to keep the PE warm through the
    final chunk's normalization latency.
  - output blocks are staged as [128, 1024] fp16 (one DMA per seq block,
    2KB lines); the final chunk's output DMAs go out on the scalar
    engine's DGE queue, which is idle once the last exp has issued, and
    its normalization broadcasts the denominator reciprocal with a K=1
    PE matmul instead of the ~1.2us GPSIMD broadcast.

Measured phases (per core): ~7us fixed engine preamble; a DMA-ramp-bound
start (the DMA path ramps from ~50 to ~400GB/s over its first ~8us; PE
warmup matmuls keep the HAM clock gate at 8/8 meanwhile); an ACT-bound
batch-0 attention phase (exp saturated, projections as filler); a
PE-bound batch-1 phase; a short drain. Run-to-run note: sustained
back-to-back launches push the chip into the P0 power state (PE drops
2.4 -> 2.0 GHz, everything measures ~20% slower) - allow a cooldown
between profiling runs.
"""

from contextlib import ExitStack

import numpy as np

import concourse.bass as bass
import concourse.mybir as mybir
import concourse.tile as tile
from concourse import bacc

F32 = mybir.dt.float32
F16 = mybir.dt.float16

B = 2
S = 2048
D = 1024
H_LOCAL = 2          # heads per core
BS = B * S           # 4096
NE = D // 128        # contraction tiles for the projections
CHUNK = 512          # query-chunk width
NCH = S // CHUNK     # chunks per batch element
NTB = S // 128       # key blocks per batch element
SCALE = 0.125        # 1/sqrt(head_dim)
VA = 80              # v_aug padded stride
N_CORES = 8
N_WARMUP = 22        # dummy matmuls to warm the PE clock gate during DMA


class _Filler:
    """Queue of deferred emission steps, drained as filler between the
    attention inner-loop iterations so the PE queue interleaves."""

    def __init__(self, steps, n_slots):
        self.steps = list(steps)
        self.per_slot = len(self.steps) / max(1, n_slots)
        self.credit = 0.0

    def tick(self):
        self.credit += self.per_slot
        while self.steps and self.credit >= 1.0:
            self.steps.pop(0)()
            self.credit -= 1.0

    def drain(self):
        while self.steps:
            self.steps.pop(0)()


def _mha_kernel(tc, out, xP, wAll):
    nc = tc.nc
    with ExitStack() as ctx:
        singles = ctx.enter_context(tc.tile_pool(name="singles", bufs=1))

        sc_ps = ctx.enter_context(
            tc.tile_pool(name="sc_ps", bufs=2, space="PSUM"))
        op_ps = ctx.enter_context(
            tc.tile_pool(name="op_ps", bufs=2, space="PSUM"))
        aux_ps = ctx.enter_context(
            tc.tile_pool(name="aux_ps", bufs=2, space="PSUM"))
        pt_pool = ctx.enter_context(tc.tile_pool(name="pt_pool", bufs=6))
        nrm = ctx.enter_context(tc.tile_pool(name="nrm", bufs=4))
        ot_pool = ctx.enter_context(tc.tile_pool(name="ot_pool", bufs=4))

        # PE warm-up: dummy matmuls issued before any data dependency so
        # the HAM clock gate reaches 8/8 while the first x DMA lands.
        dummy = singles.tile([128, CHUNK], F16, tag="dummy")
        nc.vector.memset(dummy[:], 0.0)
        for i in range(N_WARMUP):
            wps = aux_ps.tile([128, CHUNK], F32, tag="aux", name="warm")
            nc.tensor.matmul(wps[:], dummy[:, 0:128], dummy[:],
                             start=True, stop=True)

        # All input DMAs go on the sync queue in strict priority order (the
        # DMA path shares one aggregate bandwidth pool, so concurrent
        # queues would steal from the critical chunk-0 bytes). The head of
        # the stream is fine-grained so the first projection matmuls can
        # start after only ~0.75MB has landed: wq, first half of chunk-0 x,
        # wk, second half, wv+wo, then the remaining chunks in order.
        wall = singles.tile([128, 4, NE, 128], F16, tag="wall")
        xsb = singles.tile([128, 2 * NCH, NE, CHUNK], F16, tag="xsb")
        half = NE * CHUNK // 2

        def wdma(lo, hi):
            nc.sync.dma_start(
                out=wall[:, lo:hi].rearrange("p w e o -> p (w e o)"),
                in_=wAll[:, bass.ds(lo * D, (hi - lo) * D)])

        wdma(0, 1)                                            # wq
        nc.sync.dma_start(out=xsb[:, 0, 0:NE // 2],
                          in_=xP[:, bass.ds(0, half)])        # x0 e0-3
        wdma(1, 2)                                            # wk
        nc.sync.dma_start(out=xsb[:, 0, NE // 2:],
                          in_=xP[:, bass.ds(half, half)])     # x0 e4-7
        wdma(2, 4)                                            # wv, wo
        for c in range(1, 2 * NCH):
            nc.sync.dma_start(
                out=xsb[:, c], in_=xP[:, bass.ds(c * NE * CHUNK, NE * CHUNK)])
        w_sb = {"wq": wall[:, 0], "wk": wall[:, 1], "wv": wall[:, 2]}
        wo_sb = wall[:, 3].rearrange("p e o -> p (e o)")

        def xt_sl(e, c, off, width):
            """xT slice [128, width] for chunk c at column offset off."""
            return xsb[:, c, e, bass.ds(off, width)]

        ones64 = singles.tile([1, 64], F16, tag="ones64")
        nc.vector.memset(ones64[:], 1.0)
        qT = singles.tile([128, BS], F16, tag="qT")
        kT = singles.tile([128, BS], F16, tag="kT")
        v_aug = singles.tile([128, B * H_LOCAL, NTB, VA], F16, tag="v_aug")
        ones = singles.tile([128, 1], F16, tag="ones")
        nc.vector.memset(ones[:], 1.0)
        nc.vector.tensor_copy(
            v_aug[:, :, :, 64:65],
            ones[:].to_broadcast((128, B * H_LOCAL, NTB, 1)))
        y_cT = singles.tile([128, BS], F16, tag="y_cT")

        def proj_chunk_steps(c):
            """QK^T projection + direct-V for one 512-wide chunk, split into
            independently-schedulable steps."""
            cols = bass.ds(c * CHUNK, CHUNK)
            b = c // NCH
            steps = []

            def qk_step(wsb, dstT):
                ps = aux_ps.tile([128, CHUNK], F32, tag="aux", name="aux")
                for e in range(NE):
                    nc.tensor.matmul(ps[:], wsb[:, e, :],
                                     xt_sl(e, c, 0, CHUNK),
                                     start=(e == 0), stop=(e == NE - 1))
                nc.vector.tensor_copy(dstT[:, cols], ps[:])

            steps.append(lambda: qk_step(w_sb["wq"], qT))
            steps.append(lambda: qk_step(w_sb["wk"], kT))

            def v_step(jb):
                tb = (c % NCH) * (CHUNK // 128) + jb
                tr = aux_ps.tile([128, CHUNK], F32, tag="aux", name="aux")
                for e in range(NE):
                    nc.tensor.matmul(tr[:, 0:128],
                                     xt_sl(e, c, jb * 128, 128),
                                     w_sb["wv"][:, e, :],
                                     start=(e == 0), stop=(e == NE - 1))
                for h in range(H_LOCAL):
                    nc.vector.tensor_copy(
                        v_aug[:, b * H_LOCAL + h, tb, 0:64],
                        tr[:, 64 * h:64 * h + 64])

            for jb in range(CHUNK // 128):
                steps.append(lambda jb=jb: v_step(jb))
            return steps

        def attn_chunk(b, c, filler, tail=False, pre_norm_steps=()):
            """Attention for one query chunk: scores (co-issued pairs) ->
            exp (scalar engine) -> PV -> normalize. tail=True emits a
            low-latency normalization for the final chunk (og staged on the
            idle scalar engine in parallel with the reciprocal chain, and
            the broadcast done by a K=1 PE matmul - the PE is idle in the
            drain - instead of the ~1.2us GPSIMD broadcast). pre_norm_steps
            are emitted between the last PV and the normalization: filler
            that keeps the PE busy (and the HAM clock gate warm) through
            the normalization latency."""
            cols = bass.ds(c * CHUNK, CHUNK)
            with nc.named_scope(f"attn_c{c}"):
                op = [op_ps.tile([65, CHUNK], F32, tag="op", name=f"op{h}")
                      for h in range(H_LOCAL)]
                for t in range(NTB):
                    tcols = bass.ds(b * S + t * 128, 128)
                    sc = sc_ps.tile([128, H_LOCAL, CHUNK], F32, tag="sc",
                                    name="sc")
                    for h in range(H_LOCAL):
                        hp = slice(64 * h, 64 * h + 64)
                        nc.tensor.matmul(
                            sc[:, h, :], kT[hp, tcols], qT[hp, cols],
                            start=True, stop=True, tile_position=(64 * h, 0))
                    pt = pt_pool.tile([128, H_LOCAL, CHUNK], F16,
                                      tag="pt", name="pt")
                    nc.scalar.activation(
                        pt[:], sc[:], mybir.ActivationFunctionType.Exp,
                        scale=SCALE)
                    for h in range(H_LOCAL):
                        nc.tensor.matmul(
                            op[h][:], v_aug[:, b * H_LOCAL + h, t, 0:65],
                            pt[:, h, :],
                            start=(t == 0), stop=(t == NTB - 1))
                    filler.tick()
                for st in pre_norm_steps:
                    st()
                for h in range(H_LOCAL):
                    if tail:
                        og = nrm.tile([64, CHUNK], F32, tag="og")
                        nc.scalar.copy(og[:], op[h][0:64, :])
                        rs = nrm.tile([1, CHUNK], F32, tag="rs")
                        nc.vector.tensor_copy(rs[:], op[h][64:65, :])
                        rr = nrm.tile([1, CHUNK], F32, tag="rr")
                        nc.vector.reciprocal_approx_fast(out=rr[:],
                                                         in_=rs[:])
                        rrh = nrm.tile([1, CHUNK], F16, tag="rrh")
                        nc.vector.tensor_copy(rrh[:], rr[:])
                        bc_ps = aux_ps.tile([64, CHUNK], F32, tag="aux",
                                            name="bc")
                        nc.tensor.matmul(bc_ps[:], ones64[:], rrh[:],
                                         start=True, stop=True)
                        nc.vector.tensor_mul(y_cT[64 * h:64 * h + 64, cols],
                                             og[:], bc_ps[:])
                        continue
                    # og staged out of PSUM. For batch-1 chunks the scalar
                    # engine is used (it lands right behind the chunk's last
                    # exp in the ACT queue, and ACT has slack there); for
                    # batch-0 chunks ACT is the saturated engine, so the og
                    # goes to the vector engine instead.
                    og = nrm.tile([65, CHUNK], F32, tag="og")
                    if b == 0:
                        nc.vector.tensor_copy(og[:], op[h][:])
                    else:
                        nc.scalar.copy(og[:], op[h][:])
                    rs = nrm.tile([1, CHUNK], F32, tag="rs")
                    nc.vector.tensor_copy(rs[:], og[64:65, :])
                    bc = nrm.tile([64, CHUNK], F32, tag="bc")
                    nc.gpsimd.partition_broadcast(bc[:], rs[:])
                    bcr = nrm.tile([64, CHUNK], F32, tag="bcr")
                    nc.vector.reciprocal_approx_fast(out=bcr[:], in_=bc[:])
                    nc.vector.tensor_mul(y_cT[64 * h:64 * h + 64, cols],
                                         og[0:64, :], bcr[:])

        def oproj_steps(b, c, tail=False):
            """Output projection for one chunk's 4 seq-blocks; fp16 partials
            staged through SBUF as [128, 1024] blocks (host sums the 8
            cores' partials in fp32). Tail blocks DMA out on the scalar
            engine's DGE queue (idle after the last exp)."""
            steps = []

            # one matmul (+ PSUM stage) per step: fine granularity lets the
            # filler spread into the ~200ns per-key-block PE deficit of the
            # batch-1 attention phase instead of landing in 1us bursts
            def f_step(blk, f, ot_box):
                rows = bass.ds(b * S + c * CHUNK + blk * 128, 128)
                if f == 0:
                    ot_box[0] = ot_pool.tile([128, D], F16, tag="ot",
                                             name="ot")
                fcols = bass.ds(f * CHUNK, CHUNK)
                po = aux_ps.tile([128, CHUNK], F32, tag="aux", name="aux")
                nc.tensor.matmul(po[:], y_cT[:, rows], wo_sb[:, fcols],
                                 start=True, stop=True)
                # in the drain the scalar engine is idle; splitting the
                # PSUM stage across both engines keeps the PE fed
                if tail and f == 1:
                    nc.scalar.copy(ot_box[0][:, fcols], po[:])
                else:
                    nc.vector.tensor_copy(ot_box[0][:, fcols], po[:])
                if f == 1:
                    eng = nc.scalar if tail and blk % 2 == 0 else nc.sync
                    eng.dma_start(out=out[rows, :], in_=ot_box[0][:])

            for blk in range(CHUNK // 128):
                box = [None]
                for f in range(D // CHUNK):
                    steps.append(
                        lambda blk=blk, f=f, box=box: f_step(blk, f, box))
            return steps

        # ---- schedule ----------------------------------------------------
        # batch-0 projections up front (head start)
        for c in range(NCH):
            for st in proj_chunk_steps(c):
                st()

        # batch-0 attention, with batch-1 projections as PE filler
        w1 = _Filler([st for c in range(NCH, 2 * NCH)
                      for st in proj_chunk_steps(c)], NCH * NTB)
        for c in range(NCH):
            attn_chunk(0, c, w1)
        w1.drain()

        # batch-1 attention, with the output projection as PE filler:
        # a backlog of ready oproj steps is consumed with a small reserve
        # held back so the final attention chunk (which has no other
        # filler) stays fed.
        backlog = [st for c in range(NCH) for st in oproj_steps(0, c)]
        for c in range(NCH):
            last = c == NCH - 1
            take = max(0, len(backlog) - 4)
            w2 = _Filler(backlog[:take], NTB)
            pre = backlog[take:] if last else ()
            backlog = [] if last else backlog[take:]
            attn_chunk(1, NCH + c, w2, tail=last, pre_norm_steps=pre)
            w2.drain()
            backlog += oproj_steps(1, c, tail=last)
        for st in backlog:
            st()


def build_nc(n_cores=N_CORES):
    nc = bacc.Bacc("TRN2", target_bir_lowering=False, debug=False,
                   num_devices=n_cores)
    xP = nc.dram_tensor("xP", [128, 2 * NCH * NE * CHUNK], F16,
                        kind="ExternalInput").ap()
    wAll = nc.dram_tensor("wAll", [128, 4 * D], F16, kind="ExternalInput").ap()
    out = nc.dram_tensor("out", [BS, D], F16, kind="ExternalOutput").ap()
    with tile.TileContext(nc) as tc:
        _mha_kernel(tc, out, xP, wAll)
    nc.compile()
    return nc


def make_in_maps(inputs, Wq, Wk, Wv, Wo, n_cores=N_CORES):
    x = np.asarray(inputs, dtype=np.float32).reshape(BS, D)
    # [p, chunk, e, o] layout: each 512-row chunk contiguous per partition
    xP = np.ascontiguousarray(
        x.reshape(2 * NCH, CHUNK, NE, 128).transpose(3, 0, 2, 1)
    ).astype(np.float16).reshape(128, -1)
    Wq, Wk, Wv, Wo = (np.asarray(w, dtype=np.float32)
                      for w in (Wq, Wk, Wv, Wo))
    maps = []
    for c in range(n_cores):
        sl = slice(c * 128, (c + 1) * 128)
        # per-weight layout [p, e, o]: column-parallel slice, contiguous
        # 2KB-per-partition lines for the DMA
        def pre(w):
            return w.reshape(NE, 128, 128).transpose(1, 0, 2)
        wq = pre(Wq[sl, :].T)
        wk = pre(Wk[sl, :].T)
        wv = pre(Wv[sl, :].T)
        wo = Wo[:, sl].T.reshape(128, NE, 128)
        wall = np.concatenate([wq, wk, wv, wo], axis=1).astype(np.float16)
        maps.append({
            "xP": xP,
            "wAll": np.ascontiguousarray(wall.reshape(128, 4 * D)),
        })
    return maps


_NC_CACHE = None


def run(inputs, Wq, Wk, Wv, Wo, trace=False):
    """Shard, run on the 8 NeuronCores, and unshard. Returns
    (output [B,S,D] float32, BassKernelResults)."""
    global _NC_CACHE
    from concourse.bass_utils import run_bass_kernel_spmd
    if _NC_CACHE is None:
        _NC_CACHE = build_nc()
    maps = make_in_maps(inputs, Wq, Wk, Wv, Wo)
    res = run_bass_kernel_spmd(_NC_CACHE, maps, list(range(N_CORES)),
                               trace=trace)
    acc = np.zeros((BS, D), dtype=np.float32)
    for rmap in res.results:
        acc += rmap["out"].astype(np.float32)
    return acc.reshape(B, S, D), res


def kernel(inputs, Wq, Wk, Wv, Wo):
    out, _ = run(inputs, Wq, Wk, Wv, Wo, trace=False)
    return out


# revision 46
# speedup vs baseline: 1.0044x; 1.0044x over previous
"""Multi-head attention (B=2, S=2048, D=1024, H=16) on 8 TRN2 NeuronCores.

Sharding (Megatron-style): heads are tensor-parallel across the 8 cores
(2 heads each, batch replicated as part of each core's work).
Wq/Wk/Wv are column-parallel (each core gets its heads' 128 output rows),
Wo is row-parallel (each core gets the matching 128 input columns); each
core computes a full-shape partial of the output projection and the host
sums the 8 partials (the row-parallel all-reduce, done at unshard time).

Per-core kernel (fp16 matmuls, fp32 PSUM), scheduled to overlap the
scalar-engine softmax exp (the critical resource) with the tensor engine:

  - x is pre-arranged on the host so each 512-query chunk is one
    contiguous-per-partition DMA; chunks arrive in compute order so the
    first projection starts ~16us in (vs ~27us with whole-slab loads;
    the remaining latency is the DMA path's own cold ramp).
  - a short burst of dummy matmuls at t=0 warms the PE HAM clock gate
    while the first DMA is in flight.
  - V is computed directly in [seq, head-dim] layout (x^T slab as the
    stationary operand), so no PE transposes; a ones column makes the PV
    matmul also produce the softmax denominator (PSUM row 64).
  - scores are co-issued per head pair on disjoint PE row groups; exp
    reads score PSUM [128, 1024] per key block and writes fp16 P.
  - the softmax normalization stages the PV accumulator out of PSUM on
    the scalar engine (it lands right behind the chunk's last exp in the
    ACT queue, so the PSUM banks free deterministically fast), then
    normalizes on the vector engine with a GPSIMD partition-broadcast of
    the denominator row.
  - batch 1's projections fill the tensor engine during batch 0's
    attention; the output projection fills during batch 1's attention
    (emission-interleaved via _Filler so the per-engine queues overlap),
    with a couple of blocks held back to keep the PE warm through the
    final chunk's normalization latency.
  - output blocks are staged as [128, 1024] fp16 (one DMA per seq block,
    2KB lines); the final chunk's output DMAs go out on the scalar
    engine's DGE queue, which is idle once the last exp has issued, and
    its normalization broadcasts the denominator reciprocal with a K=1
    PE matmul instead of the ~1.2us GPSIMD broadcast.

Measured phases (per core): ~7us fixed engine preamble; a DMA-ramp-bound
start (the DMA path ramps from ~50 to ~400GB/s over its first ~8us; PE
warmup matmuls keep the HAM clock gate at 8/8 meanwhile); an ACT-bound
batch-0 attention phase (exp saturated, projections as filler); a
PE-bound batch-1 phase; a short drain. Run-to-run note: sustained
back-to-back launches push the chip into the P0 power state (PE drops
2.4 -> 2.0 GHz, everything measures ~20% slower) - allow a cooldown
between profiling runs.
"""

from contextlib import ExitStack

import numpy as np

import concourse.bass as bass
import concourse.mybir as mybir
import concourse.tile as tile
from concourse import bacc

F32 = mybir.dt.float32
F16 = mybir.dt.float16

B = 2
S = 2048
D = 1024
H_LOCAL = 2          # heads per core
BS = B * S           # 4096
NE = D // 128        # contraction tiles for the projections
CHUNK = 512          # query-chunk width
NCH = S // CHUNK     # chunks per batch element
NTB = S // 128       # key blocks per batch element
SCALE = 0.125        # 1/sqrt(head_dim)
VA = 80              # v_aug padded stride
N_CORES = 8
N_WARMUP = 22        # dummy matmuls to warm the PE clock gate during DMA


class _Filler:
    """Queue of deferred emission steps, drained as filler between the
    attention inner-loop iterations so the PE queue interleaves."""

    def __init__(self, steps, n_slots):
        self.steps = list(steps)
        self.per_slot = len(self.steps) / max(1, n_slots)
        self.credit = 0.0

    def tick(self):
        self.credit += self.per_slot
        while self.steps and self.credit >= 1.0:
            self.steps.pop(0)()
            self.credit -= 1.0

    def drain(self):
        while self.steps:
            self.steps.pop(0)()


def _mha_kernel(tc, out, xP, wAll):
    nc = tc.nc
    with ExitStack() as ctx:
        singles = ctx.enter_context(tc.tile_pool(name="singles", bufs=1))

        sc_ps = ctx.enter_context(
            tc.tile_pool(name="sc_ps", bufs=2, space="PSUM"))
        op_ps = ctx.enter_context(
            tc.tile_pool(name="op_ps", bufs=2, space="PSUM"))
        aux_ps = ctx.enter_context(
            tc.tile_pool(name="aux_ps", bufs=2, space="PSUM"))
        pt_pool = ctx.enter_context(tc.tile_pool(name="pt_pool", bufs=6))
        nrm = ctx.enter_context(tc.tile_pool(name="nrm", bufs=4))
        ot_pool = ctx.enter_context(tc.tile_pool(name="ot_pool", bufs=4))

        # PE warm-up: dummy matmuls issued before any data dependency so
        # the HAM clock gate reaches 8/8 while the first x DMA lands.
        dummy = singles.tile([128, CHUNK], F16, tag="dummy")
        nc.vector.memset(dummy[:], 0.0)
        for i in range(N_WARMUP):
            wps = aux_ps.tile([128, CHUNK], F32, tag="aux", name="warm")
            nc.tensor.matmul(wps[:], dummy[:, 0:128], dummy[:],
                             start=True, stop=True)

        # All input DMAs go on the sync queue in strict priority order (the
        # DMA path shares one aggregate bandwidth pool, so concurrent
        # queues would steal from the critical chunk-0 bytes). The head of
        # the stream is fine-grained so the first projection matmuls can
        # start after only ~0.75MB has landed: wq, first half of chunk-0 x,
        # wk, second half, wv+wo, then the remaining chunks in order.
        wall = singles.tile([128, 4, NE, 128], F16, tag="wall")
        xsb = singles.tile([128, 2 * NCH, NE, CHUNK], F16, tag="xsb")
        half = NE * CHUNK // 2

        def wdma(lo, hi):
            nc.sync.dma_start(
                out=wall[:, lo:hi].rearrange("p w e o -> p (w e o)"),
                in_=wAll[:, bass.ds(lo * D, (hi - lo) * D)])

        wdma(0, 1)                                            # wq
        nc.sync.dma_start(out=xsb[:, 0, 0:NE // 2],
                          in_=xP[:, bass.ds(0, half)])        # x0 e0-3
        wdma(1, 2)                                            # wk
        nc.sync.dma_start(out=xsb[:, 0, NE // 2:],
                          in_=xP[:, bass.ds(half, half)])     # x0 e4-7
        wdma(2, 4)                                            # wv, wo
        for c in range(1, 2 * NCH):
            nc.sync.dma_start(
                out=xsb[:, c], in_=xP[:, bass.ds(c * NE * CHUNK, NE * CHUNK)])
        w_sb = {"wq": wall[:, 0], "wk": wall[:, 1], "wv": wall[:, 2]}
        wo_sb = wall[:, 3].rearrange("p e o -> p (e o)")

        def xt_sl(e, c, off, width):
            """xT slice [128, width] for chunk c at column offset off."""
            return xsb[:, c, e, bass.ds(off, width)]

        ones64 = singles.tile([1, 64], F16, tag="ones64")
        nc.vector.memset(ones64[:], 1.0)
        qT = singles.tile([128, BS], F16, tag="qT")
        kT = singles.tile([128, BS], F16, tag="kT")
        v_aug = singles.tile([128, B * H_LOCAL, NTB, VA], F16, tag="v_aug")
        ones = singles.tile([128, 1], F16, tag="ones")
        nc.vector.memset(ones[:], 1.0)
        nc.vector.tensor_copy(
            v_aug[:, :, :, 64:65],
            ones[:].to_broadcast((128, B * H_LOCAL, NTB, 1)))
        y_cT = singles.tile([128, BS], F16, tag="y_cT")

        def proj_chunk_steps(c):
            """QK^T projection + direct-V for one 512-wide chunk, split into
            independently-schedulable steps."""
            cols = bass.ds(c * CHUNK, CHUNK)
            b = c // NCH
            steps = []

            def qk_step(wsb, dstT):
                ps = aux_ps.tile([128, CHUNK], F32, tag="aux", name="aux")
                for e in range(NE):
                    nc.tensor.matmul(ps[:], wsb[:, e, :],
                                     xt_sl(e, c, 0, CHUNK),
                                     start=(e == 0), stop=(e == NE - 1))
                nc.vector.tensor_copy(dstT[:, cols], ps[:])

            steps.append(lambda: qk_step(w_sb["wq"], qT))
            steps.append(lambda: qk_step(w_sb["wk"], kT))

            def v_step(jb):
                tb = (c % NCH) * (CHUNK // 128) + jb
                tr = aux_ps.tile([128, CHUNK], F32, tag="aux", name="aux")
                for e in range(NE):
                    nc.tensor.matmul(tr[:, 0:128],
                                     xt_sl(e, c, jb * 128, 128),
                                     w_sb["wv"][:, e, :],
                                     start=(e == 0), stop=(e == NE - 1))
                for h in range(H_LOCAL):
                    nc.vector.tensor_copy(
                        v_aug[:, b * H_LOCAL + h, tb, 0:64],
                        tr[:, 64 * h:64 * h + 64])

            for jb in range(CHUNK // 128):
                steps.append(lambda jb=jb: v_step(jb))
            return steps

        def attn_chunk(b, c, filler, tail=False, pre_norm_steps=()):
            """Attention for one query chunk: scores (co-issued pairs) ->
            exp (scalar engine) -> PV -> normalize. tail=True emits a
            low-latency normalization for the final chunk (og staged on the
            idle scalar engine in parallel with the reciprocal chain, and
            the broadcast done by a K=1 PE matmul - the PE is idle in the
            drain - instead of the ~1.2us GPSIMD broadcast). pre_norm_steps
            are emitted between the last PV and the normalization: filler
            that keeps the PE busy (and the HAM clock gate warm) through
            the normalization latency."""
            cols = bass.ds(c * CHUNK, CHUNK)
            with nc.named_scope(f"attn_c{c}"):
                op = [op_ps.tile([65, CHUNK], F32, tag="op", name=f"op{h}")
                      for h in range(H_LOCAL)]
                for t in range(NTB):
                    tcols = bass.ds(b * S + t * 128, 128)
                    sc = sc_ps.tile([128, H_LOCAL, CHUNK], F32, tag="sc",
                                    name="sc")
                    for h in range(H_LOCAL):
                        hp = slice(64 * h, 64 * h + 64)
                        nc.tensor.matmul(
                            sc[:, h, :], kT[hp, tcols], qT[hp, cols],
                            start=True, stop=True, tile_position=(64 * h, 0))
                    pt = pt_pool.tile([128, H_LOCAL, CHUNK], F16,
                                      tag="pt", name="pt")
                    nc.scalar.activation(
                        pt[:], sc[:], mybir.ActivationFunctionType.Exp,
                        scale=SCALE)
                    for h in range(H_LOCAL):
                        nc.tensor.matmul(
                            op[h][:], v_aug[:, b * H_LOCAL + h, t, 0:65],
                            pt[:, h, :],
                            start=(t == 0), stop=(t == NTB - 1))
                    filler.tick()
                for st in pre_norm_steps:
                    st()
                for h in range(H_LOCAL):
                    if tail:
                        og = nrm.tile([64, CHUNK], F32, tag="og")
                        nc.scalar.copy(og[:], op[h][0:64, :])
                        rs = nrm.tile([1, CHUNK], F32, tag="rs")
                        nc.vector.tensor_copy(rs[:], op[h][64:65, :])
                        rr = nrm.tile([1, CHUNK], F32, tag="rr")
                        nc.vector.reciprocal_approx_fast(out=rr[:],
                                                         in_=rs[:])
                        rrh = nrm.tile([1, CHUNK], F16, tag="rrh")
                        nc.vector.tensor_copy(rrh[:], rr[:])
                        bc_ps = aux_ps.tile([64, CHUNK], F32, tag="aux",
                                            name="bc")
                        nc.tensor.matmul(bc_ps[:], ones64[:], rrh[:],
                                         start=True, stop=True)
                        nc.vector.tensor_mul(y_cT[64 * h:64 * h + 64, cols],
                                             og[:], bc_ps[:])
                        continue
                    # og staged out of PSUM. For batch-1 chunks the scalar
                    # engine is used (it lands right behind the chunk's last
                    # exp in the ACT queue, and ACT has slack there); for
                    # batch-0 chunks ACT is the saturated engine, so the og
                    # goes to the vector engine instead.
                    og = nrm.tile([65, CHUNK], F32, tag="og")
                    if b == 0:
                        nc.vector.tensor_copy(og[:], op[h][:])
                    else:
                        nc.scalar.copy(og[:], op[h][:])
                    rs = nrm.tile([1, CHUNK], F32, tag="rs")
                    nc.vector.tensor_copy(rs[:], og[64:65, :])
                    bc = nrm.tile([64, CHUNK], F32, tag="bc")
                    nc.gpsimd.partition_broadcast(bc[:], rs[:])
                    bcr = nrm.tile([64, CHUNK], F32, tag="bcr")
                    nc.vector.reciprocal_approx_fast(out=bcr[:], in_=bc[:])
                    nc.vector.tensor_mul(y_cT[64 * h:64 * h + 64, cols],
                                         og[0:64, :], bcr[:])

        def oproj_steps(b, c, tail=False):
            """Output projection for one chunk's 4 seq-blocks; fp16 partials
            staged through SBUF as [128, 1024] blocks (host sums the 8
            cores' partials in fp32). Tail blocks DMA out on the scalar
            engine's DGE queue (idle after the last exp)."""
            steps = []

            def blk_step(blk):
                rows = bass.ds(b * S + c * CHUNK + blk * 128, 128)
                ot = ot_pool.tile([128, D], F16, tag="ot", name="ot")
                for f in range(D // CHUNK):
                    fcols = bass.ds(f * CHUNK, CHUNK)
                    po = aux_ps.tile([128, CHUNK], F32, tag="aux", name="aux")
                    nc.tensor.matmul(po[:], y_cT[:, rows], wo_sb[:, fcols],
                                     start=True, stop=True)
                    # in the drain the scalar engine is idle; splitting the
                    # PSUM stage across both engines keeps the PE fed
                    if tail and f == 1:
                        nc.scalar.copy(ot[:, fcols], po[:])
                    else:
                        nc.vector.tensor_copy(ot[:, fcols], po[:])
                eng = nc.scalar if tail and blk % 2 == 0 else nc.sync
                eng.dma_start(out=out[rows, :], in_=ot[:])

            # paired 2-block steps: 4 back-to-back matmuls amortize the
            # stationary-switch entry cost (these run in the PE-bound
            # batch-1 phase, where the scalar engine has slack)
            def pair(b0, b1):
                blk_step(b0)
                blk_step(b1)

            steps.append(lambda: pair(0, 1))
            steps.append(lambda: pair(2, 3))
            return steps

        # ---- schedule ----------------------------------------------------
        # batch-0 projections up front (head start)
        for c in range(NCH):
            for st in proj_chunk_steps(c):
                st()

        # batch-0 attention, with batch-1 projections as PE filler
        w1 = _Filler([st for c in range(NCH, 2 * NCH)
                      for st in proj_chunk_steps(c)], NCH * NTB)
        for c in range(NCH):
            attn_chunk(0, c, w1)
        w1.drain()

        # batch-1 attention, with the output projection as PE filler:
        # a backlog of ready oproj steps is consumed with a small reserve
        # held back so the final attention chunk (which has no other
        # filler) stays fed.
        backlog = [st for c in range(NCH) for st in oproj_steps(0, c)]
        for c in range(NCH):
            last = c == NCH - 1
            take = max(0, len(backlog) - 2)
            w2 = _Filler(backlog[:take], NTB)
            pre = backlog[take:] if last else ()
            backlog = [] if last else backlog[take:]
            attn_chunk(1, NCH + c, w2, tail=last, pre_norm_steps=pre)
            w2.drain()
            backlog += oproj_steps(1, c, tail=last)
        for st in backlog:
            st()


def build_nc(n_cores=N_CORES):
    nc = bacc.Bacc("TRN2", target_bir_lowering=False, debug=False,
                   num_devices=n_cores)
    xP = nc.dram_tensor("xP", [128, 2 * NCH * NE * CHUNK], F16,
                        kind="ExternalInput").ap()
    wAll = nc.dram_tensor("wAll", [128, 4 * D], F16, kind="ExternalInput").ap()
    out = nc.dram_tensor("out", [BS, D], F16, kind="ExternalOutput").ap()
    with tile.TileContext(nc) as tc:
        _mha_kernel(tc, out, xP, wAll)
    nc.compile()
    return nc


def make_in_maps(inputs, Wq, Wk, Wv, Wo, n_cores=N_CORES):
    x = np.asarray(inputs, dtype=np.float32).reshape(BS, D)
    # [p, chunk, e, o] layout: each 512-row chunk contiguous per partition
    xP = np.ascontiguousarray(
        x.reshape(2 * NCH, CHUNK, NE, 128).transpose(3, 0, 2, 1)
    ).astype(np.float16).reshape(128, -1)
    Wq, Wk, Wv, Wo = (np.asarray(w, dtype=np.float32)
                      for w in (Wq, Wk, Wv, Wo))
    maps = []
    for c in range(n_cores):
        sl = slice(c * 128, (c + 1) * 128)
        # per-weight layout [p, e, o]: column-parallel slice, contiguous
        # 2KB-per-partition lines for the DMA
        def pre(w):
            return w.reshape(NE, 128, 128).transpose(1, 0, 2)
        wq = pre(Wq[sl, :].T)
        wk = pre(Wk[sl, :].T)
        wv = pre(Wv[sl, :].T)
        wo = Wo[:, sl].T.reshape(128, NE, 128)
        wall = np.concatenate([wq, wk, wv, wo], axis=1).astype(np.float16)
        maps.append({
            "xP": xP,
            "wAll": np.ascontiguousarray(wall.reshape(128, 4 * D)),
        })
    return maps


_NC_CACHE = None


def run(inputs, Wq, Wk, Wv, Wo, trace=False):
    """Shard, run on the 8 NeuronCores, and unshard. Returns
    (output [B,S,D] float32, BassKernelResults)."""
    global _NC_CACHE
    from concourse.bass_utils import run_bass_kernel_spmd
    if _NC_CACHE is None:
        _NC_CACHE = build_nc()
    maps = make_in_maps(inputs, Wq, Wk, Wv, Wo)
    res = run_bass_kernel_spmd(_NC_CACHE, maps, list(range(N_CORES)),
                               trace=trace)
    acc = np.zeros((BS, D), dtype=np.float32)
    for rmap in res.results:
        acc += rmap["out"].astype(np.float32)
    return acc.reshape(B, S, D), res


def kernel(inputs, Wq, Wk, Wv, Wo):
    out, _ = run(inputs, Wq, Wk, Wv, Wo, trace=False)
    return out


# revision 48
# speedup vs baseline: 1.0139x; 1.0094x over previous
"""Multi-head attention (B=2, S=2048, D=1024, H=16) on 8 TRN2 NeuronCores.

Sharding (Megatron-style): heads are tensor-parallel across the 8 cores
(2 heads each, batch replicated as part of each core's work).
Wq/Wk/Wv are column-parallel (each core gets its heads' 128 output rows),
Wo is row-parallel (each core gets the matching 128 input columns); each
core computes a full-shape partial of the output projection and the host
sums the 8 partials (the row-parallel all-reduce, done at unshard time).

Per-core kernel (fp16 matmuls, fp32 PSUM), scheduled to overlap the
scalar-engine softmax exp (the critical resource) with the tensor engine:

  - x is pre-arranged on the host so each 512-query chunk is one
    contiguous-per-partition DMA; chunks arrive in compute order so the
    first projection starts ~16us in (vs ~27us with whole-slab loads;
    the remaining latency is the DMA path's own cold ramp).
  - a short burst of dummy matmuls at t=0 warms the PE HAM clock gate
    while the first DMA is in flight.
  - V is computed directly in [seq, head-dim] layout (x^T slab as the
    stationary operand), so no PE transposes; a ones column makes the PV
    matmul also produce the softmax denominator (PSUM row 64).
  - scores are co-issued per head pair on disjoint PE row groups; exp
    reads score PSUM [128, 1024] per key block and writes fp16 P.
  - the softmax normalization stages the PV accumulator out of PSUM on
    the scalar engine (it lands right behind the chunk's last exp in the
    ACT queue, so the PSUM banks free deterministically fast), then
    normalizes on the vector engine with a GPSIMD partition-broadcast of
    the denominator row.
  - batch 1's projections fill the tensor engine during batch 0's
    attention; the output projection fills during batch 1's attention
    (emission-interleaved via _Filler so the per-engine queues overlap),
    with a couple of blocks held back to keep the PE warm through the
    final chunk's normalization latency.
  - output blocks are staged as [128, 1024] fp16 (one DMA per seq block,
    2KB lines); the final chunk's output DMAs go out on the scalar
    engine's DGE queue, which is idle once the last exp has issued, and
    its normalization broadcasts the denominator reciprocal with a K=1
    PE matmul instead of the ~1.2us GPSIMD broadcast.

Measured phases (per core): ~7us fixed engine preamble; a DMA-ramp-bound
start (the DMA path ramps from ~50 to ~400GB/s over its first ~8us; PE
warmup matmuls keep the HAM clock gate at 8/8 meanwhile); an ACT-bound
batch-0 attention phase (exp saturated, projections as filler); a
PE-bound batch-1 phase; a short drain. Run-to-run note: sustained
back-to-back launches push the chip into the P0 power state (PE drops
2.4 -> 2.0 GHz, everything measures ~20% slower) - allow a cooldown
between profiling runs.
"""

from contextlib import ExitStack

import numpy as np

import concourse.bass as bass
import concourse.mybir as mybir
import concourse.tile as tile
from concourse import bacc

F32 = mybir.dt.float32
F16 = mybir.dt.float16

B = 2
S = 2048
D = 1024
H_LOCAL = 2          # heads per core
BS = B * S           # 4096
NE = D // 128        # contraction tiles for the projections
CHUNK = 512          # query-chunk width
NCH = S // CHUNK     # chunks per batch element
NTB = S // 128       # key blocks per batch element
SCALE = 0.125        # 1/sqrt(head_dim)
VA = 80              # v_aug padded stride
N_CORES = 8
N_WARMUP = 22        # dummy matmuls to warm the PE clock gate during DMA


class _Filler:
    """Queue of deferred emission steps, drained as filler between the
    attention inner-loop iterations so the PE queue interleaves."""

    def __init__(self, steps, n_slots):
        self.steps = list(steps)
        self.per_slot = len(self.steps) / max(1, n_slots)
        self.credit = 0.0

    def tick(self):
        self.credit += self.per_slot
        while self.steps and self.credit >= 1.0:
            self.steps.pop(0)()
            self.credit -= 1.0

    def drain(self):
        while self.steps:
            self.steps.pop(0)()


def _mha_kernel(tc, out, xP, wAll):
    nc = tc.nc
    with ExitStack() as ctx:
        singles = ctx.enter_context(tc.tile_pool(name="singles", bufs=1))

        sc_ps = ctx.enter_context(
            tc.tile_pool(name="sc_ps", bufs=2, space="PSUM"))
        op_ps = ctx.enter_context(
            tc.tile_pool(name="op_ps", bufs=2, space="PSUM"))
        aux_ps = ctx.enter_context(
            tc.tile_pool(name="aux_ps", bufs=2, space="PSUM"))
        pt_pool = ctx.enter_context(tc.tile_pool(name="pt_pool", bufs=6))
        nrm = ctx.enter_context(tc.tile_pool(name="nrm", bufs=4))
        ot_pool = ctx.enter_context(tc.tile_pool(name="ot_pool", bufs=4))

        # PE warm-up: dummy matmuls issued before any data dependency so
        # the HAM clock gate reaches 8/8 while the first x DMA lands.
        dummy = singles.tile([128, CHUNK], F16, tag="dummy")
        nc.vector.memset(dummy[:], 0.0)
        for i in range(N_WARMUP):
            wps = aux_ps.tile([128, CHUNK], F32, tag="aux", name="warm")
            nc.tensor.matmul(wps[:], dummy[:, 0:128], dummy[:],
                             start=True, stop=True)

        # All input DMAs go on the sync queue in strict priority order (the
        # DMA path shares one aggregate bandwidth pool, so concurrent
        # queues would steal from the critical chunk-0 bytes). The head of
        # the stream is fine-grained so the first projection matmuls can
        # start after only ~0.75MB has landed: wq, first half of chunk-0 x,
        # wk, second half, wv+wo, then the remaining chunks in order.
        wall = singles.tile([128, 4, NE, 128], F16, tag="wall")
        xsb = singles.tile([128, 2 * NCH, NE, CHUNK], F16, tag="xsb")
        half = NE * CHUNK // 2

        def wdma(lo, hi):
            nc.sync.dma_start(
                out=wall[:, lo:hi].rearrange("p w e o -> p (w e o)"),
                in_=wAll[:, bass.ds(lo * D, (hi - lo) * D)])

        wdma(0, 1)                                            # wq
        nc.sync.dma_start(out=xsb[:, 0, 0:NE // 2],
                          in_=xP[:, bass.ds(0, half)])        # x0 e0-3
        wdma(1, 2)                                            # wk
        nc.sync.dma_start(out=xsb[:, 0, NE // 2:],
                          in_=xP[:, bass.ds(half, half)])     # x0 e4-7
        wdma(2, 4)                                            # wv, wo
        for c in range(1, 2 * NCH):
            nc.sync.dma_start(
                out=xsb[:, c], in_=xP[:, bass.ds(c * NE * CHUNK, NE * CHUNK)])
        w_sb = {"wq": wall[:, 0], "wk": wall[:, 1], "wv": wall[:, 2]}
        wo_sb = wall[:, 3].rearrange("p e o -> p (e o)")

        def xt_sl(e, c, off, width):
            """xT slice [128, width] for chunk c at column offset off."""
            return xsb[:, c, e, bass.ds(off, width)]

        ones64 = singles.tile([1, 64], F16, tag="ones64")
        nc.vector.memset(ones64[:], 1.0)
        qT = singles.tile([128, BS], F16, tag="qT")
        kT = singles.tile([128, BS], F16, tag="kT")
        v_aug = singles.tile([128, B * H_LOCAL, NTB, VA], F16, tag="v_aug")
        ones = singles.tile([128, 1], F16, tag="ones")
        nc.vector.memset(ones[:], 1.0)
        nc.vector.tensor_copy(
            v_aug[:, :, :, 64:65],
            ones[:].to_broadcast((128, B * H_LOCAL, NTB, 1)))
        y_cT = singles.tile([128, BS], F16, tag="y_cT")

        def proj_chunk_steps(c):
            """QK^T projection + direct-V for one 512-wide chunk, split into
            independently-schedulable steps."""
            cols = bass.ds(c * CHUNK, CHUNK)
            b = c // NCH
            steps = []

            def qk_step(wsb, dstT):
                ps = aux_ps.tile([128, CHUNK], F32, tag="aux", name="aux")
                for e in range(NE):
                    nc.tensor.matmul(ps[:], wsb[:, e, :],
                                     xt_sl(e, c, 0, CHUNK),
                                     start=(e == 0), stop=(e == NE - 1))
                nc.vector.tensor_copy(dstT[:, cols], ps[:])

            steps.append(lambda: qk_step(w_sb["wq"], qT))
            steps.append(lambda: qk_step(w_sb["wk"], kT))

            def v_step(jb):
                tb = (c % NCH) * (CHUNK // 128) + jb
                tr = aux_ps.tile([128, CHUNK], F32, tag="aux", name="aux")
                for e in range(NE):
                    nc.tensor.matmul(tr[:, 0:128],
                                     xt_sl(e, c, jb * 128, 128),
                                     w_sb["wv"][:, e, :],
                                     start=(e == 0), stop=(e == NE - 1))
                for h in range(H_LOCAL):
                    nc.vector.tensor_copy(
                        v_aug[:, b * H_LOCAL + h, tb, 0:64],
                        tr[:, 64 * h:64 * h + 64])

            for jb in range(CHUNK // 128):
                steps.append(lambda jb=jb: v_step(jb))
            return steps

        def attn_chunk(b, c, filler, tail=False, pre_norm_steps=()):
            """Attention for one query chunk: scores (co-issued pairs) ->
            exp (scalar engine) -> PV -> normalize. tail=True emits a
            low-latency normalization for the final chunk (og staged on the
            idle scalar engine in parallel with the reciprocal chain, and
            the broadcast done by a K=1 PE matmul - the PE is idle in the
            drain - instead of the ~1.2us GPSIMD broadcast). pre_norm_steps
            are emitted between the last PV and the normalization: filler
            that keeps the PE busy (and the HAM clock gate warm) through
            the normalization latency."""
            cols = bass.ds(c * CHUNK, CHUNK)
            with nc.named_scope(f"attn_c{c}"):
                op = [op_ps.tile([65, CHUNK], F32, tag="op", name=f"op{h}")
                      for h in range(H_LOCAL)]
                for t in range(NTB):
                    tcols = bass.ds(b * S + t * 128, 128)
                    sc = sc_ps.tile([128, H_LOCAL, CHUNK], F32, tag="sc",
                                    name="sc")
                    for h in range(H_LOCAL):
                        hp = slice(64 * h, 64 * h + 64)
                        nc.tensor.matmul(
                            sc[:, h, :], kT[hp, tcols], qT[hp, cols],
                            start=True, stop=True, tile_position=(64 * h, 0))
                    pt = pt_pool.tile([128, H_LOCAL, CHUNK], F16,
                                      tag="pt", name="pt")
                    nc.scalar.activation(
                        pt[:], sc[:], mybir.ActivationFunctionType.Exp,
                        scale=SCALE)
                    for h in range(H_LOCAL):
                        nc.tensor.matmul(
                            op[h][:], v_aug[:, b * H_LOCAL + h, t, 0:65],
                            pt[:, h, :],
                            start=(t == 0), stop=(t == NTB - 1))
                    filler.tick()
                for st in pre_norm_steps:
                    st()
                for h in range(H_LOCAL):
                    if tail:
                        og = nrm.tile([64, CHUNK], F32, tag="og")
                        nc.scalar.copy(og[:], op[h][0:64, :])
                        rs = nrm.tile([1, CHUNK], F32, tag="rs")
                        nc.vector.tensor_copy(rs[:], op[h][64:65, :])
                        rr = nrm.tile([1, CHUNK], F32, tag="rr")
                        nc.vector.reciprocal_approx_fast(out=rr[:],
                                                         in_=rs[:])
                        rrh = nrm.tile([1, CHUNK], F16, tag="rrh")
                        nc.vector.tensor_copy(rrh[:], rr[:])
                        bc_ps = aux_ps.tile([64, CHUNK], F32, tag="aux",
                                            name="bc")
                        nc.tensor.matmul(bc_ps[:], ones64[:], rrh[:],
                                         start=True, stop=True)
                        nc.vector.tensor_mul(y_cT[64 * h:64 * h + 64, cols],
                                             og[:], bc_ps[:])
                        continue
                    # og staged out of PSUM. For batch-1 chunks the scalar
                    # engine is used (it lands right behind the chunk's last
                    # exp in the ACT queue, and ACT has slack there); for
                    # batch-0 chunks ACT is the saturated engine, so the og
                    # goes to the vector engine instead.
                    og = nrm.tile([65, CHUNK], F32, tag="og")
                    if b == 0:
                        nc.vector.tensor_copy(og[:], op[h][:])
                    else:
                        nc.scalar.copy(og[:], op[h][:])
                    rs = nrm.tile([1, CHUNK], F32, tag="rs")
                    nc.vector.tensor_copy(rs[:], og[64:65, :])
                    bc = nrm.tile([64, CHUNK], F32, tag="bc")
                    nc.gpsimd.partition_broadcast(bc[:], rs[:])
                    bcr = nrm.tile([64, CHUNK], F32, tag="bcr")
                    nc.vector.reciprocal_approx_fast(out=bcr[:], in_=bc[:])
                    nc.vector.tensor_mul(y_cT[64 * h:64 * h + 64, cols],
                                         og[0:64, :], bcr[:])

        def oproj_steps(b, c, tail=False):
            """Output projection for one chunk's 4 seq-blocks; fp16 partials
            staged through SBUF as [128, 1024] blocks (host sums the 8
            cores' partials in fp32). Tail blocks DMA out on the scalar
            engine's DGE queue (idle after the last exp)."""
            steps = []

            def blk_step(blk):
                rows = bass.ds(b * S + c * CHUNK + blk * 128, 128)
                ot = ot_pool.tile([128, D], F16, tag="ot", name="ot")
                for f in range(D // CHUNK):
                    fcols = bass.ds(f * CHUNK, CHUNK)
                    # drain blocks alternate between the aux pool and the
                    # just-released op accumulator banks: a 4-deep PSUM
                    # rotation across the two pools lets the matmuls run at
                    # stream rate instead of waiting on each block's cast
                    pool = op_ps if tail and f == 1 else aux_ps
                    po = pool.tile([128, CHUNK], F32, tag="op" if pool is
                                   op_ps else "aux", name="aux")
                    nc.tensor.matmul(po[:], y_cT[:, rows], wo_sb[:, fcols],
                                     start=True, stop=True)
                    # in the drain the scalar engine is idle; splitting the
                    # PSUM stage across both engines keeps the PE fed
                    if tail and f == 1:
                        nc.scalar.copy(ot[:, fcols], po[:])
                    else:
                        nc.vector.tensor_copy(ot[:, fcols], po[:])
                eng = nc.scalar if tail and blk % 2 == 0 else nc.sync
                eng.dma_start(out=out[rows, :], in_=ot[:])

            # paired 2-block steps: 4 back-to-back matmuls amortize the
            # stationary-switch entry cost (these run in the PE-bound
            # batch-1 phase, where the scalar engine has slack)
            def pair(b0, b1):
                blk_step(b0)
                blk_step(b1)

            steps.append(lambda: pair(0, 1))
            steps.append(lambda: pair(2, 3))
            return steps

        # ---- schedule ----------------------------------------------------
        # batch-0 projections up front (head start)
        for c in range(NCH):
            for st in proj_chunk_steps(c):
                st()

        # batch-0 attention, with batch-1 projections as PE filler
        w1 = _Filler([st for c in range(NCH, 2 * NCH)
                      for st in proj_chunk_steps(c)], NCH * NTB)
        for c in range(NCH):
            attn_chunk(0, c, w1)
        w1.drain()

        # batch-1 attention, with the output projection as PE filler:
        # a backlog of ready oproj steps is consumed with a small reserve
        # held back so the final attention chunk (which has no other
        # filler) stays fed.
        backlog = [st for c in range(NCH) for st in oproj_steps(0, c)]
        for c in range(NCH):
            last = c == NCH - 1
            take = max(0, len(backlog) - 2)
            w2 = _Filler(backlog[:take], NTB)
            pre = backlog[take:] if last else ()
            backlog = [] if last else backlog[take:]
            attn_chunk(1, NCH + c, w2, tail=last, pre_norm_steps=pre)
            w2.drain()
            backlog += oproj_steps(1, c, tail=last)
        for st in backlog:
            st()


def build_nc(n_cores=N_CORES):
    nc = bacc.Bacc("TRN2", target_bir_lowering=False, debug=False,
                   num_devices=n_cores)
    xP = nc.dram_tensor("xP", [128, 2 * NCH * NE * CHUNK], F16,
                        kind="ExternalInput").ap()
    wAll = nc.dram_tensor("wAll", [128, 4 * D], F16, kind="ExternalInput").ap()
    out = nc.dram_tensor("out", [BS, D], F16, kind="ExternalOutput").ap()
    with tile.TileContext(nc) as tc:
        _mha_kernel(tc, out, xP, wAll)
    nc.compile()
    return nc


def make_in_maps(inputs, Wq, Wk, Wv, Wo, n_cores=N_CORES):
    x = np.asarray(inputs, dtype=np.float32).reshape(BS, D)
    # [p, chunk, e, o] layout: each 512-row chunk contiguous per partition
    xP = np.ascontiguousarray(
        x.reshape(2 * NCH, CHUNK, NE, 128).transpose(3, 0, 2, 1)
    ).astype(np.float16).reshape(128, -1)
    Wq, Wk, Wv, Wo = (np.asarray(w, dtype=np.float32)
                      for w in (Wq, Wk, Wv, Wo))
    maps = []
    for c in range(n_cores):
        sl = slice(c * 128, (c + 1) * 128)
        # per-weight layout [p, e, o]: column-parallel slice, contiguous
        # 2KB-per-partition lines for the DMA
        def pre(w):
            return w.reshape(NE, 128, 128).transpose(1, 0, 2)
        wq = pre(Wq[sl, :].T)
        wk = pre(Wk[sl, :].T)
        wv = pre(Wv[sl, :].T)
        wo = Wo[:, sl].T.reshape(128, NE, 128)
        wall = np.concatenate([wq, wk, wv, wo], axis=1).astype(np.float16)
        maps.append({
            "xP": xP,
            "wAll": np.ascontiguousarray(wall.reshape(128, 4 * D)),
        })
    return maps


_NC_CACHE = None


def run(inputs, Wq, Wk, Wv, Wo, trace=False):
    """Shard, run on the 8 NeuronCores, and unshard. Returns
    (output [B,S,D] float32, BassKernelResults)."""
    global _NC_CACHE
    from concourse.bass_utils import run_bass_kernel_spmd
    if _NC_CACHE is None:
        _NC_CACHE = build_nc()
    maps = make_in_maps(inputs, Wq, Wk, Wv, Wo)
    res = run_bass_kernel_spmd(_NC_CACHE, maps, list(range(N_CORES)),
                               trace=trace)
    acc = np.zeros((BS, D), dtype=np.float32)
    for rmap in res.results:
        acc += rmap["out"].astype(np.float32)
    return acc.reshape(B, S, D), res


def kernel(inputs, Wq, Wk, Wv, Wo):
    out, _ = run(inputs, Wq, Wk, Wv, Wo, trace=False)
    return out


# revision 49
# speedup vs baseline: 1.0271x; 1.0131x over previous
"""Multi-head attention (B=2, S=2048, D=1024, H=16) on 8 TRN2 NeuronCores.

Sharding (Megatron-style): heads are tensor-parallel across the 8 cores
(2 heads each, batch replicated as part of each core's work).
Wq/Wk/Wv are column-parallel (each core gets its heads' 128 output rows),
Wo is row-parallel (each core gets the matching 128 input columns); each
core computes a full-shape partial of the output projection and the host
sums the 8 partials (the row-parallel all-reduce, done at unshard time).

Per-core kernel (fp16 matmuls, fp32 PSUM), scheduled to overlap the
scalar-engine softmax exp (the critical resource) with the tensor engine:

  - x is pre-arranged on the host so each 512-query chunk is one
    contiguous-per-partition DMA; chunks arrive in compute order so the
    first projection starts ~16us in (vs ~27us with whole-slab loads;
    the remaining latency is the DMA path's own cold ramp).
  - a short burst of dummy matmuls at t=0 warms the PE HAM clock gate
    while the first DMA is in flight.
  - V is computed directly in [seq, head-dim] layout (x^T slab as the
    stationary operand), so no PE transposes; a ones column makes the PV
    matmul also produce the softmax denominator (PSUM row 64).
  - scores are co-issued per head pair on disjoint PE row groups; exp
    reads score PSUM [128, 1024] per key block and writes fp16 P.
  - the softmax normalization stages the PV accumulator out of PSUM on
    the scalar engine (it lands right behind the chunk's last exp in the
    ACT queue, so the PSUM banks free deterministically fast), then
    normalizes on the vector engine with a GPSIMD partition-broadcast of
    the denominator row.
  - batch 1's projections fill the tensor engine during batch 0's
    attention; the output projection fills during batch 1's attention
    (emission-interleaved via _Filler so the per-engine queues overlap),
    with a couple of blocks held back to keep the PE warm through the
    final chunk's normalization latency.
  - output blocks are staged as [128, 1024] fp16 (one DMA per seq block,
    2KB lines); the final chunk's output DMAs go out on the scalar
    engine's DGE queue, which is idle once the last exp has issued, and
    its normalization broadcasts the denominator reciprocal with a K=1
    PE matmul instead of the ~1.2us GPSIMD broadcast.

Measured phases (per core): ~7us fixed engine preamble; a DMA-ramp-bound
start (the DMA path ramps from ~50 to ~400GB/s over its first ~8us; PE
warmup matmuls keep the HAM clock gate at 8/8 meanwhile); an ACT-bound
batch-0 attention phase (exp saturated, projections as filler); a
PE-bound batch-1 phase; a short drain. Run-to-run note: sustained
back-to-back launches push the chip into the P0 power state (PE drops
2.4 -> 2.0 GHz, everything measures ~20% slower) - allow a cooldown
between profiling runs.
"""

from contextlib import ExitStack

import numpy as np

import concourse.bass as bass
import concourse.mybir as mybir
import concourse.tile as tile
from concourse import bacc

F32 = mybir.dt.float32
F16 = mybir.dt.float16

B = 2
S = 2048
D = 1024
H_LOCAL = 2          # heads per core
BS = B * S           # 4096
NE = D // 128        # contraction tiles for the projections
CHUNK = 512          # query-chunk width
NCH = S // CHUNK     # chunks per batch element
NTB = S // 128       # key blocks per batch element
SCALE = 0.125        # 1/sqrt(head_dim)
VA = 80              # v_aug padded stride
N_CORES = 8
N_WARMUP = 22        # dummy matmuls to warm the PE clock gate during DMA


class _Filler:
    """Queue of deferred emission steps, drained as filler between the
    attention inner-loop iterations so the PE queue interleaves."""

    def __init__(self, steps, n_slots):
        self.steps = list(steps)
        self.per_slot = len(self.steps) / max(1, n_slots)
        self.credit = 0.0

    def tick(self):
        self.credit += self.per_slot
        while self.steps and self.credit >= 1.0:
            self.steps.pop(0)()
            self.credit -= 1.0

    def drain(self):
        while self.steps:
            self.steps.pop(0)()


def _mha_kernel(tc, out, xP, wAll):
    nc = tc.nc
    with ExitStack() as ctx:
        singles = ctx.enter_context(tc.tile_pool(name="singles", bufs=1))

        sc_ps = ctx.enter_context(
            tc.tile_pool(name="sc_ps", bufs=2, space="PSUM"))
        op_ps = ctx.enter_context(
            tc.tile_pool(name="op_ps", bufs=2, space="PSUM"))
        aux_ps = ctx.enter_context(
            tc.tile_pool(name="aux_ps", bufs=2, space="PSUM"))
        pt_pool = ctx.enter_context(tc.tile_pool(name="pt_pool", bufs=6))
        nrm = ctx.enter_context(tc.tile_pool(name="nrm", bufs=4))
        ot_pool = ctx.enter_context(tc.tile_pool(name="ot_pool", bufs=4))

        # PE warm-up: dummy matmuls issued before any data dependency so
        # the HAM clock gate reaches 8/8 while the first x DMA lands.
        dummy = singles.tile([128, CHUNK], F16, tag="dummy")
        nc.vector.memset(dummy[:], 0.0)
        for i in range(N_WARMUP):
            wps = aux_ps.tile([128, CHUNK], F32, tag="aux", name="warm")
            nc.tensor.matmul(wps[:], dummy[:, 0:128], dummy[:],
                             start=True, stop=True)

        # All input DMAs go on the sync queue in strict priority order (the
        # DMA path shares one aggregate bandwidth pool, so concurrent
        # queues would steal from the critical chunk-0 bytes). The head of
        # the stream is fine-grained so the first projection matmuls can
        # start after only ~0.75MB has landed: wq, first half of chunk-0 x,
        # wk, second half, wv+wo, then the remaining chunks in order.
        wall = singles.tile([128, 4, NE, 128], F16, tag="wall")
        xsb = singles.tile([128, 2 * NCH, NE, CHUNK], F16, tag="xsb")
        half = NE * CHUNK // 2

        def wdma(lo, hi):
            nc.sync.dma_start(
                out=wall[:, lo:hi].rearrange("p w e o -> p (w e o)"),
                in_=wAll[:, bass.ds(lo * D, (hi - lo) * D)])

        wdma(0, 1)                                            # wq
        nc.sync.dma_start(out=xsb[:, 0, 0:NE // 2],
                          in_=xP[:, bass.ds(0, half)])        # x0 e0-3
        wdma(1, 2)                                            # wk
        nc.sync.dma_start(out=xsb[:, 0, NE // 2:],
                          in_=xP[:, bass.ds(half, half)])     # x0 e4-7
        wdma(2, 4)                                            # wv, wo
        for c in range(1, 2 * NCH):
            nc.sync.dma_start(
                out=xsb[:, c], in_=xP[:, bass.ds(c * NE * CHUNK, NE * CHUNK)])
        w_sb = {"wq": wall[:, 0], "wk": wall[:, 1], "wv": wall[:, 2]}
        wo_sb = wall[:, 3].rearrange("p e o -> p (e o)")

        def xt_sl(e, c, off, width):
            """xT slice [128, width] for chunk c at column offset off."""
            return xsb[:, c, e, bass.ds(off, width)]

        ones64 = singles.tile([1, 64], F16, tag="ones64")
        nc.vector.memset(ones64[:], 1.0)
        qT = singles.tile([128, BS], F16, tag="qT")
        kT = singles.tile([128, BS], F16, tag="kT")
        v_aug = singles.tile([128, B * H_LOCAL, NTB, VA], F16, tag="v_aug")
        ones = singles.tile([128, 1], F16, tag="ones")
        nc.vector.memset(ones[:], 1.0)
        nc.vector.tensor_copy(
            v_aug[:, :, :, 64:65],
            ones[:].to_broadcast((128, B * H_LOCAL, NTB, 1)))
        y_cT = singles.tile([128, BS], F16, tag="y_cT")

        def proj_chunk_steps(c):
            """QK^T projection + direct-V for one 512-wide chunk, split into
            independently-schedulable steps."""
            cols = bass.ds(c * CHUNK, CHUNK)
            b = c // NCH
            steps = []

            def qk_step(wsb, dstT):
                ps = aux_ps.tile([128, CHUNK], F32, tag="aux", name="aux")
                for e in range(NE):
                    nc.tensor.matmul(ps[:], wsb[:, e, :],
                                     xt_sl(e, c, 0, CHUNK),
                                     start=(e == 0), stop=(e == NE - 1))
                nc.vector.tensor_copy(dstT[:, cols], ps[:])

            steps.append(lambda: qk_step(w_sb["wq"], qT))
            steps.append(lambda: qk_step(w_sb["wk"], kT))

            def v_step(jb):
                tb = (c % NCH) * (CHUNK // 128) + jb
                tr = aux_ps.tile([128, CHUNK], F32, tag="aux", name="aux")
                for e in range(NE):
                    nc.tensor.matmul(tr[:, 0:128],
                                     xt_sl(e, c, jb * 128, 128),
                                     w_sb["wv"][:, e, :],
                                     start=(e == 0), stop=(e == NE - 1))
                for h in range(H_LOCAL):
                    nc.vector.tensor_copy(
                        v_aug[:, b * H_LOCAL + h, tb, 0:64],
                        tr[:, 64 * h:64 * h + 64])

            for jb in range(CHUNK // 128):
                steps.append(lambda jb=jb: v_step(jb))
            return steps

        def attn_chunk(b, c, filler, tail=False, pre_norm_steps=()):
            """Attention for one query chunk: scores (co-issued pairs) ->
            exp (scalar engine) -> PV -> normalize. tail=True emits a
            low-latency normalization for the final chunk (og staged on the
            idle scalar engine in parallel with the reciprocal chain, and
            the broadcast done by a K=1 PE matmul - the PE is idle in the
            drain - instead of the ~1.2us GPSIMD broadcast). pre_norm_steps
            are emitted between the last PV and the normalization: filler
            that keeps the PE busy (and the HAM clock gate warm) through
            the normalization latency."""
            cols = bass.ds(c * CHUNK, CHUNK)
            with nc.named_scope(f"attn_c{c}"):
                op = [op_ps.tile([65, CHUNK], F32, tag="op", name=f"op{h}")
                      for h in range(H_LOCAL)]
                for t in range(NTB):
                    tcols = bass.ds(b * S + t * 128, 128)
                    sc = sc_ps.tile([128, H_LOCAL, CHUNK], F32, tag="sc",
                                    name="sc")
                    for h in range(H_LOCAL):
                        hp = slice(64 * h, 64 * h + 64)
                        nc.tensor.matmul(
                            sc[:, h, :], kT[hp, tcols], qT[hp, cols],
                            start=True, stop=True, tile_position=(64 * h, 0))
                    pt = pt_pool.tile([128, H_LOCAL, CHUNK], F16,
                                      tag="pt", name="pt")
                    nc.scalar.activation(
                        pt[:], sc[:], mybir.ActivationFunctionType.Exp,
                        scale=SCALE)
                    for h in range(H_LOCAL):
                        nc.tensor.matmul(
                            op[h][:], v_aug[:, b * H_LOCAL + h, t, 0:65],
                            pt[:, h, :],
                            start=(t == 0), stop=(t == NTB - 1))
                    filler.tick()
                for st in pre_norm_steps:
                    st()
                for h in range(H_LOCAL):
                    if tail:
                        og = nrm.tile([64, CHUNK], F32, tag="og")
                        nc.scalar.copy(og[:], op[h][0:64, :])
                        rs = nrm.tile([1, CHUNK], F32, tag="rs")
                        nc.vector.tensor_copy(rs[:], op[h][64:65, :])
                        rr = nrm.tile([1, CHUNK], F32, tag="rr")
                        nc.vector.reciprocal_approx_fast(out=rr[:],
                                                         in_=rs[:])
                        rrh = nrm.tile([1, CHUNK], F16, tag="rrh")
                        nc.vector.tensor_copy(rrh[:], rr[:])
                        bc_ps = aux_ps.tile([64, CHUNK], F32, tag="aux",
                                            name="bc")
                        nc.tensor.matmul(bc_ps[:], ones64[:], rrh[:],
                                         start=True, stop=True)
                        nc.vector.tensor_mul(y_cT[64 * h:64 * h + 64, cols],
                                             og[:], bc_ps[:])
                        continue
                    # og staged out of PSUM. For batch-1 chunks the scalar
                    # engine is used (it lands right behind the chunk's last
                    # exp in the ACT queue, and ACT has slack there); for
                    # batch-0 chunks ACT is the saturated engine, so the og
                    # goes to the vector engine instead.
                    og = nrm.tile([65, CHUNK], F32, tag="og")
                    if b == 0:
                        nc.vector.tensor_copy(og[:], op[h][:])
                    else:
                        nc.scalar.copy(og[:], op[h][:])
                    rs = nrm.tile([1, CHUNK], F32, tag="rs")
                    nc.vector.tensor_copy(rs[:], og[64:65, :])
                    bc = nrm.tile([64, CHUNK], F32, tag="bc")
                    nc.gpsimd.partition_broadcast(bc[:], rs[:])
                    bcr = nrm.tile([64, CHUNK], F32, tag="bcr")
                    nc.vector.reciprocal_approx_fast(out=bcr[:], in_=bc[:])
                    nc.vector.tensor_mul(y_cT[64 * h:64 * h + 64, cols],
                                         og[0:64, :], bcr[:])

        def oproj_steps(b, c, tail=False):
            """Output projection for one chunk's 4 seq-blocks; fp16 partials
            staged through SBUF as [128, 1024] blocks (host sums the 8
            cores' partials in fp32). Tail blocks DMA out on the scalar
            engine's DGE queue (idle after the last exp)."""
            steps = []

            def blk_step(blk):
                rows = bass.ds(b * S + c * CHUNK + blk * 128, 128)
                ot = ot_pool.tile([128, D], F16, tag="ot", name="ot")
                for f in range(D // CHUNK):
                    fcols = bass.ds(f * CHUNK, CHUNK)
                    # drain blocks alternate between the aux pool and the
                    # just-released op accumulator banks: a 4-deep PSUM
                    # rotation across the two pools lets the matmuls run at
                    # stream rate instead of waiting on each block's cast
                    pool = op_ps if tail and f == 1 else aux_ps
                    po = pool.tile([128, CHUNK], F32, tag="op" if pool is
                                   op_ps else "aux", name="aux")
                    nc.tensor.matmul(po[:], y_cT[:, rows], wo_sb[:, fcols],
                                     start=True, stop=True)
                    # in the drain the scalar engine is idle; splitting the
                    # PSUM stage across both engines keeps the PE fed
                    if tail and f == 1:
                        nc.scalar.copy(ot[:, fcols], po[:])
                    else:
                        nc.vector.tensor_copy(ot[:, fcols], po[:])
                eng = nc.scalar if tail and blk % 2 == 0 else nc.sync
                eng.dma_start(out=out[rows, :], in_=ot[:])

            # paired 2-block steps: 4 back-to-back matmuls amortize the
            # stationary-switch entry cost (these run in the PE-bound
            # batch-1 phase, where the scalar engine has slack)
            def pair(b0, b1):
                blk_step(b0)
                blk_step(b1)

            steps.append(lambda: pair(0, 1))
            steps.append(lambda: pair(2, 3))
            return steps

        # ---- schedule ----------------------------------------------------
        # batch-0 projections up front (head start)
        for c in range(NCH):
            for st in proj_chunk_steps(c):
                st()

        # batch-0 attention, with batch-1 projections as PE filler
        w1 = _Filler([st for c in range(NCH, 2 * NCH)
                      for st in proj_chunk_steps(c)], NCH * NTB)
        for c in range(NCH):
            attn_chunk(0, c, w1)
        w1.drain()

        # batch-1 attention, with the output projection as PE filler:
        # a backlog of ready oproj steps is consumed with a small reserve
        # held back so the final attention chunk (which has no other
        # filler) stays fed.
        backlog = [st for c in range(NCH) for st in oproj_steps(0, c)]
        for c in range(NCH):
            last = c == NCH - 1
            take = max(0, len(backlog) - 4)
            w2 = _Filler(backlog[:take], NTB)
            pre = backlog[take:] if last else ()
            backlog = [] if last else backlog[take:]
            attn_chunk(1, NCH + c, w2, tail=last, pre_norm_steps=pre)
            w2.drain()
            backlog += oproj_steps(1, c, tail=last)
        for st in backlog:
            st()


def build_nc(n_cores=N_CORES):
    nc = bacc.Bacc("TRN2", target_bir_lowering=False, debug=False,
                   num_devices=n_cores)
    xP = nc.dram_tensor("xP", [128, 2 * NCH * NE * CHUNK], F16,
                        kind="ExternalInput").ap()
    wAll = nc.dram_tensor("wAll", [128, 4 * D], F16, kind="ExternalInput").ap()
    out = nc.dram_tensor("out", [BS, D], F16, kind="ExternalOutput").ap()
    with tile.TileContext(nc) as tc:
        _mha_kernel(tc, out, xP, wAll)
    nc.compile()
    return nc


def make_in_maps(inputs, Wq, Wk, Wv, Wo, n_cores=N_CORES):
    x = np.asarray(inputs, dtype=np.float32).reshape(BS, D)
    # [p, chunk, e, o] layout: each 512-row chunk contiguous per partition
    xP = np.ascontiguousarray(
        x.reshape(2 * NCH, CHUNK, NE, 128).transpose(3, 0, 2, 1)
    ).astype(np.float16).reshape(128, -1)
    Wq, Wk, Wv, Wo = (np.asarray(w, dtype=np.float32)
                      for w in (Wq, Wk, Wv, Wo))
    maps = []
    for c in range(n_cores):
        sl = slice(c * 128, (c + 1) * 128)
        # per-weight layout [p, e, o]: column-parallel slice, contiguous
        # 2KB-per-partition lines for the DMA
        def pre(w):
            return w.reshape(NE, 128, 128).transpose(1, 0, 2)
        wq = pre(Wq[sl, :].T)
        wk = pre(Wk[sl, :].T)
        wv = pre(Wv[sl, :].T)
        wo = Wo[:, sl].T.reshape(128, NE, 128)
        wall = np.concatenate([wq, wk, wv, wo], axis=1).astype(np.float16)
        maps.append({
            "xP": xP,
            "wAll": np.ascontiguousarray(wall.reshape(128, 4 * D)),
        })
    return maps


_NC_CACHE = None


def run(inputs, Wq, Wk, Wv, Wo, trace=False):
    """Shard, run on the 8 NeuronCores, and unshard. Returns
    (output [B,S,D] float32, BassKernelResults)."""
    global _NC_CACHE
    from concourse.bass_utils import run_bass_kernel_spmd
    if _NC_CACHE is None:
        _NC_CACHE = build_nc()
    maps = make_in_maps(inputs, Wq, Wk, Wv, Wo)
    res = run_bass_kernel_spmd(_NC_CACHE, maps, list(range(N_CORES)),
                               trace=trace)
    acc = np.zeros((BS, D), dtype=np.float32)
    for rmap in res.results:
        acc += rmap["out"].astype(np.float32)
    return acc.reshape(B, S, D), res


def kernel(inputs, Wq, Wk, Wv, Wo):
    out, _ = run(inputs, Wq, Wk, Wv, Wo, trace=False)
    return out
